# revision 56
# baseline (speedup 1.0000x reference)
"""Cross-modal attention (CMAttention) Trainium2 kernel.

Strategy: 8-way SPMD over (batch=4) x (modality=2). After the reference's
concat([q_x, q_a]) and 8-head split with head_dim=128, heads 0-3 depend only
on modality x and heads 4-7 only on modality a.  Each core therefore owns one
(batch, modality) pair end-to-end with zero communication:
  QKV projection (bf16 matmul) -> LayerNorm on q,k (bn_stats) -> RoPE
  (table multiplies) -> per-head DMA-transpose of q,k to [d, tok] ->
  scores^T matmul -> exp on ScalarE (scale folded) -> attn @ [v | 1]
  (ones column yields the softmax denominator for free) -> normalize.

v2 surgical changes over the original schedule:
  - W chunks load on the scalar HWDGE ring (they used to sit behind ~8us
    of GpSimd ring DRAINs, delaying the first matmul to 13us).
  - ~4us of dummy matmuls warm the PE HAM clock gate during the input
    load wait, so the first QKV matmuls run at full clock.
  - outputs DMA per (head, qc) 128x128 block on the GpSimd ring right
    after normalization instead of per-qc after the last head, cutting
    the ~13us output tail to ~1us.
"""

import os
import sys

for _p in ("/opt/trn_rl_repo", os.path.expanduser("~/.axon_site/_ro/trn_rl_repo")):
    if os.path.isdir(_p) and _p not in sys.path:
        sys.path.append(_p)

from contextlib import ExitStack

import ml_dtypes
import numpy as np

import concourse.bacc as bacc
import concourse.bass as bass
import concourse.mybir as mybir
import concourse.tile as tile
from concourse.bass_utils import run_bass_kernel_spmd

if os.environ.get("K_LDWOPT"):
    import concourse.bass_utils as _bu

    _orig_run_command = _bu.run_command

    def _patched_run_command(argv, **kw):
        argv = [
            "--enable-ldw-opt=true" if a == "--enable-ldw-opt=false" else a
            for a in argv
        ]
        return _orig_run_command(argv, **kw)

    _bu.run_command = _patched_run_command

BF16 = mybir.dt.float16
F32 = mybir.dt.float32
NPBF16 = np.float16

DIM = 512          # per-modality feature dim
N_TOK = 1024       # sequence length
NH = 4             # heads handled per core (one modality's heads)
D = 128            # head dim
NT = 8             # token tiles of 128
EPS = 1e-5
SCALE = 1.0 / float(np.sqrt(D))
VW = 132           # per-head v block width: 128 d + 1 ones + 3 pad
AF = mybir.ActivationFunctionType


def _load_tiled(nc, sbuf_tile, dram, blocks):
    """DMA a [blocks*128, C] DRAM tensor into a [128, blocks, C] SBUF tile.
    GpSimd's SWDGE ring handles the strided pattern efficiently."""
    nc.gpsimd.dma_start(
        out=sbuf_tile, in_=dram.ap().rearrange("(a b) c -> b a c", b=128)
    )


def build_module(trivial: bool):
    """Build the per-core Bass program.  trivial=True assumes all LN gains are
    exactly 1 and biases exactly 0 (folded tables are plain cos/sin and the
    additive rope term vanishes); trivial=False uses full-width tables with
    g folded in and an extra additive T3 table.

    Pipeline: stage A (qkv matmul -> LN -> rope -> spill to DRAM) runs per
    128-token tile; DMA-transposes then produce q/k in [d, tok] layout and
    stage B scores/exp/AV run per head, AV of head h overlapping the scores
    of head h+1."""
    nc = bacc.Bacc("TRN2", target_bir_lowering=False, debug=False, num_devices=8)

    xT = nc.dram_tensor("xT", [DIM, N_TOK], BF16, kind="ExternalInput")
    W = nc.dram_tensor("W", [DIM, 3 * DIM], BF16, kind="ExternalInput")
    if trivial:
        T1 = nc.dram_tensor("T1", [N_TOK, 64], BF16, kind="ExternalInput")
        T2N = nc.dram_tensor("T2N", [N_TOK, 64], BF16, kind="ExternalInput")
        T2P = nc.dram_tensor("T2P", [N_TOK, 64], BF16, kind="ExternalInput")
    else:
        T1 = nc.dram_tensor("T1", [N_TOK, 1024], BF16, kind="ExternalInput")
        T2 = nc.dram_tensor("T2", [N_TOK, 1024], BF16, kind="ExternalInput")
        T3 = nc.dram_tensor("T3", [N_TOK, 1024], BF16, kind="ExternalInput")
    out_d = nc.dram_tensor("out", [N_TOK, DIM], F32, kind="ExternalOutput")

    with tile.TileContext(nc) as tc, ExitStack() as ctx:
        consts = ctx.enter_context(tc.tile_pool(name="consts", bufs=1))
        small = ctx.enter_context(tc.tile_pool(name="small", bufs=4))
        upool = ctx.enter_context(tc.tile_pool(name="upool", bufs=3))
        # 4 bufs: the SWDGE stores that consume m1/m2 drain slowly; 2 bufs
        # made the next tile's rope mul wait on the store (WAR) of t-2
        rpool = ctx.enter_context(tc.tile_pool(name="rpool", bufs=4))
        epool = ctx.enter_context(tc.tile_pool(name="epool", bufs=2))
        dpool = ctx.enter_context(tc.tile_pool(name="dpool", bufs=1, space="DRAM"))
        psum_qk = ctx.enter_context(tc.tile_pool(name="psqk", bufs=3, space="PSUM"))
        psum_v = ctx.enter_context(tc.tile_pool(name="psv", bufs=2, space="PSUM"))

        # ---- constants: xT on the sync ring, W on the scalar ring (the
        # GpSimd ring opens with ~8us of DRAINs; keeping W off it lets the
        # first matmul start as soon as xT0+W0 land) ----
        xr = xT.ap().rearrange("(a b) c -> b a c", b=128)
        wr = W.ap().rearrange("(a b) c -> b a c", b=128)
        xT_k, W_k = [], []
        for kc in range(4):
            xt = consts.tile([128, N_TOK], BF16, name=f"xT{kc}", tag=f"xT{kc}")
            nc.sync.dma_start(out=xt, in_=xr[:, kc])
            xT_k.append(xt)
            wt = consts.tile([128, 3 * DIM], BF16, name=f"W{kc}", tag=f"W{kc}")
            nc.scalar.dma_start(out=wt, in_=wr[:, kc])
            W_k.append(wt)
        if trivial:
            cos_sb = consts.tile([128, NT, 64], BF16, tag="cos")
            _load_tiled(nc, cos_sb, T1, NT)
            # combined [-sin | +sin] table so the swap-half rope multiply is
            # one full-width op (paired with a negative-stride view of u)
            sinNP_sb = consts.tile([128, NT, 2, 64], BF16, tag="sinNP")
            _load_tiled(nc, sinNP_sb[:, :, 0, :], T2N, NT)
            _load_tiled(nc, sinNP_sb[:, :, 1, :], T2P, NT)
        else:
            T1_sb = consts.tile([128, NT, 1024], BF16, tag="T1")
            _load_tiled(nc, T1_sb, T1, NT)
            T2_sb = consts.tile([128, NT, 1024], BF16, tag="T2")
            _load_tiled(nc, T2_sb, T2, NT)
            T3_sb = consts.tile([128, NT, 1024], BF16, tag="T3")
            _load_tiled(nc, T3_sb, T3, NT)
        eps_sb = consts.tile([128, 1], F32, tag="eps")
        nc.vector.memset(eps_sb, EPS)
        # eps*D: bias for the k-side sqrt that folds 1/sqrt(D) into rstd_k
        eps128_sb = consts.tile([128, 1], F32, tag="eps128")
        nc.vector.memset(eps128_sb, EPS * D)
        # rsk_all[:, t] = rstd_k * SCALE for k-tile t (feeds the exp scale)
        rsk_all = consts.tile([128, NT], F32, tag="rsk")

        # warm the PE HAM clock gate (~4us of dummy matmuls) during the
        # input-load wait so the first QKV matmuls run at full clock
        dummy = consts.tile([128, 64], F32, tag="dummy")
        nc.vector.memset(dummy, 0.0)
        warm_ps = psum_v.tile([128, DIM], F32, tag="v", name="warm_ps", bufs=2)
        for _ in range(36):
            nc.tensor.matmul(
                warm_ps[0:64, 0:64], lhsT=dummy, rhs=dummy,
                start=True, stop=True,
            )

        v_sb = consts.tile([128, NT, NH, VW], BF16, tag="v")
        nc.vector.memset(v_sb[:, :, :, 128:129], 1.0)

        qkT_sb = [
            [
                consts.tile(
                    [128, N_TOK], BF16, name=f"qkT{s}{h}", tag=f"qkT{s}{h}"
                )
                for h in range(NH)
            ]
            for s in range(2)
        ]
        out_sb = consts.tile([128, NT, DIM], F32, tag="osb")
        r_dram = dpool.tile([N_TOK, 2 * DIM], BF16, name="r_dram", tag="r_dram")

        def bcast(ap2d, dims):
            """[128, 64] AP -> [128, *dims, 64] with stride-0 broadcast dims."""
            p, last = ap2d.ap[0], ap2d.ap[-1]
            return bass.AP(
                tensor=ap2d.tensor,
                offset=ap2d.offset,
                ap=[p] + [[0, d] for d in dims] + [last],
            )

        def half(ap, i):
            return ap.rearrange("p (b half j) -> p b half j", half=2, j=64)[
                :, :, i, :
            ]

        # ---------------- stage A: one 128-token tile ----------------
        u_of = {}

        def stage_a1(t):
            qkv_ps = psum_qk.tile([128, 2 * DIM], F32, tag="qk", name="qkv_ps")
            v_ps = psum_v.tile([128, DIM], F32, tag="v", name="v_ps")
            # all q/k matmuls first: the LN stats only read qkv_ps, so they
            # fire 4 matmuls earlier than with v interleaved
            for kc in range(4):
                for j in range(2):
                    nc.tensor.matmul(
                        qkv_ps[:, j * 512 : (j + 1) * 512],
                        lhsT=xT_k[kc][:, t * 128 : (t + 1) * 128],
                        rhs=W_k[kc][:, j * 512 : (j + 1) * 512],
                        start=(kc == 0),
                        stop=(kc == 3),
                    )
            for kc in range(4):
                nc.tensor.matmul(
                    v_ps,
                    lhsT=xT_k[kc][:, t * 128 : (t + 1) * 128],
                    rhs=W_k[kc][:, 1024:1536],
                    start=(kc == 0),
                    stop=(kc == 3),
                )

            u = upool.tile([128, 2 * DIM], BF16, tag="u", name="u")
            mvs = []
            for s in range(2):
                st = small.tile([128, 6], F32, tag=f"st{s}", name="st")
                nc.vector.bn_stats(out=st, in_=qkv_ps[:, s * 512 : (s + 1) * 512])
                mv = small.tile([128, 2], F32, tag=f"mv{s}", name="mv")
                nc.vector.bn_aggr(out=mv, in_=st)
                mvs.append(mv)
            mv_q, mv_k = mvs

            # q half: full LN apply
            sd_q = small.tile([128, 1], F32, tag="sdq", name="sd_q")
            nc.scalar.activation(sd_q, mv_q[:, 1:2], AF.Sqrt, bias=eps_sb)
            rstd_q = small.tile([128, 1], F32, tag="rsq", name="rstd_q")
            nc.vector.reciprocal(rstd_q, sd_q)
            nmr_q = small.tile([128, 1], F32, tag="nmq", name="nmr_q")
            nc.vector.scalar_tensor_tensor(
                out=nmr_q, in0=mv_q[:, 0:1], scalar=-1.0, in1=rstd_q,
                op0=mybir.AluOpType.mult, op1=mybir.AluOpType.mult,
            )
            nc.scalar.activation(
                out=u[:, 0:512], in_=qkv_ps[:, 0:512],
                func=AF.Identity, scale=rstd_q, bias=nmr_q,
            )

            # k half: sigma deferred into the exp scale (trivial g=1,b=0
            # path only); u_k = k - mu_k and rsk = rstd_k/sqrt(D)
            if trivial:
                sd2_k = small.tile([128, 1], F32, tag="sdk", name="sd2_k")
                nc.scalar.activation(
                    sd2_k, mv_k[:, 1:2], AF.Sqrt, bias=eps128_sb, scale=float(D)
                )
                negmu_k = small.tile([128, 1], F32, tag="nmk", name="negmu_k")
                nc.vector.tensor_scalar_mul(negmu_k, mv_k[:, 0:1], -1.0)
                nc.vector.reciprocal(rsk_all[:, t : t + 1], sd2_k)
                nc.scalar.activation(
                    out=u[:, 512:1024], in_=qkv_ps[:, 512:1024],
                    func=AF.Identity, scale=1.0, bias=negmu_k,
                )
            else:
                sd_k = small.tile([128, 1], F32, tag="sdk", name="sd_k")
                nc.scalar.activation(sd_k, mv_k[:, 1:2], AF.Sqrt, bias=eps_sb)
                rstd_k = small.tile([128, 1], F32, tag="rsk2", name="rstd_k")
                nc.vector.reciprocal(rstd_k, sd_k)
                nmr_k = small.tile([128, 1], F32, tag="nmk2", name="nmr_k")
                nc.vector.scalar_tensor_tensor(
                    out=nmr_k, in0=mv_k[:, 0:1], scalar=-1.0, in1=rstd_k,
                    op0=mybir.AluOpType.mult, op1=mybir.AluOpType.mult,
                )
                nc.scalar.activation(
                    out=u[:, 512:1024], in_=qkv_ps[:, 512:1024],
                    func=AF.Identity, scale=rstd_k, bias=nmr_k,
                )

            # v (raw) into augmented per-head layout, on ScalarE (VectorE is
            # the stage-A pacing engine at ~5us/tile; ScalarE has slack)
            nc.scalar.activation(
                out=v_sb[:, t, :, 0:128],
                in_=v_ps.rearrange("p (h d) -> p h d", h=NH),
                func=AF.Copy,
            )

            u_of[t] = u

        def stage_a2(t):
            u = u_of.pop(t)
            # rope: r = u * T1 + swap_half(u) * T2 (+ T3); summed on VectorE
            # and spilled on the sync ring (an accumulating SWDGE store was
            # tried and lagged ~10us behind, delaying the transposes)
            m1 = rpool.tile([128, 2 * DIM], BF16, tag="m1", name="m1")
            m2 = rpool.tile([128, 2 * DIM], BF16, tag="m2", name="m2")
            r = rpool.tile([128, 2 * DIM], BF16, tag="r", name="r")
            if trivial:
                nc.vector.tensor_mul(
                    m1.rearrange("p (b j) -> p b j", j=64),
                    u.rearrange("p (b j) -> p b j", j=64),
                    bcast(cos_sb[:, t], (16,)),
                )
                # m2 = swap_half(u) * [-sin|+sin] in ONE full-width op: a
                # negative-stride view of u swaps the 64-halves in place
                u4 = u.rearrange("p (b half j) -> p b half j", half=2, j=64)
                u_swap = bass.AP(
                    tensor=u4.tensor,
                    offset=u4.offset + 64,
                    ap=[u4.ap[0], u4.ap[1], [-64, 2], [1, 64]],
                )
                snp = sinNP_sb[:, t]  # [128, 2, 64]
                snp_b = bass.AP(
                    tensor=snp.tensor,
                    offset=snp.offset,
                    ap=[snp.ap[0], [0, 8], snp.ap[1], snp.ap[2]],
                )
                nc.vector.tensor_mul(
                    m2.rearrange("p (b half j) -> p b half j", half=2, j=64),
                    u_swap,
                    snp_b,
                )
                nc.vector.tensor_add(r, m1, m2)
            else:
                t1v = T1_sb[:, t]
                t2v = T2_sb[:, t]
                t3v = T3_sb[:, t]
                nc.vector.tensor_mul(m1, u, t1v)
                nc.vector.tensor_mul(half(m2, 0), half(u, 1), half(t2v, 0))
                nc.vector.tensor_mul(half(m2, 1), half(u, 0), half(t2v, 1))
                nc.vector.tensor_add(m1, m1, m2)
                nc.vector.tensor_add(r, m1, t3v)
            nc.sync.dma_start(out=r_dram[t * 128 : (t + 1) * 128, :], in_=r)

        def transpose_head(h):
            for s in range(2):
                blk = (s * NH + h) * 128
                # head 0's k-transpose rides the scalar HWDGE ring so q0/k0
                # transpose in parallel -- they gate the first score matmul
                ring = nc.scalar if (h == 0 and s == 1) else nc.sync
                ring.dma_start(
                    out=qkT_sb[s][h],
                    in_=r_dram[:, blk : blk + 128],
                    transpose=True,
                )

        # ---------------- stage B pieces ----------------
        # exp tiles: ets[h][kc] = exp(scores^T) [128 k, 1024 q] bf16
        ets = [[None] * NT for _ in range(NH)]

        def scores_kc(h, kc):
            qT, kT = qkT_sb[0][h], qkT_sb[1][h]
            sc_ps = psum_qk.tile([128, 2 * DIM], F32, tag="qk", name="sc_ps")
            for qh in range(2):
                nc.tensor.matmul(
                    sc_ps[:, qh * 512 : (qh + 1) * 512],
                    lhsT=kT[:, kc * 128 : (kc + 1) * 128],
                    rhs=qT[:, qh * 512 : (qh + 1) * 512],
                    start=True,
                    stop=True,
                )
            et = epool.tile(
                [128, N_TOK], BF16, tag=f"exp{h}_{kc}",
                name=f"exp{h}_{kc}", bufs=1,
            )
            nc.scalar.activation(
                out=et,
                in_=sc_ps[:, 0:N_TOK],
                func=AF.Exp,
                scale=rsk_all[:, kc : kc + 1] if trivial else SCALE,
            )
            ets[h][kc] = et

        def emit_av(h):
            for qc in range(NT):
                av = psum_v.tile([128, VW], F32, tag="v", name="av")
                for kc in range(NT):
                    et = ets[h][kc]
                    nc.tensor.matmul(
                        av[:, 0:129],
                        lhsT=et[:, qc * 128 : (qc + 1) * 128],
                        rhs=v_sb[:, kc, h, 0:129],
                        start=(kc == 0),
                        stop=(kc == NT - 1),
                    )
                rcp = small.tile([128, 1], F32, tag="rcp", name="rcp")
                nc.vector.reciprocal(rcp, av[:, 128:129])
                dst = out_sb[:, qc, h * 128 : (h + 1) * 128]
                if h == NH - 1:
                    nc.scalar.activation(
                        dst, av[:, 0:128],
                        AF.Copy, scale=rcp,
                    )
                else:
                    nc.vector.tensor_scalar_mul(dst, av[:, 0:128], rcp)
                # per-(head, qc) output block; odd heads (incl the last) on
                # the sync ring, which is idle by then -- the tail after the
                # last AV shrinks to ~1us and avoids the SWDGE drain backlog
                ring = nc.sync if h % 2 else nc.gpsimd
                ring.dma_start(
                    out=out_d.ap()[
                        qc * 128 : (qc + 1) * 128, h * 128 : (h + 1) * 128
                    ],
                    in_=dst,
                )

        # ---------------- emission schedule ----------------
        stage_a1(0)
        for t in range(1, NT):
            stage_a1(t)
            stage_a2(t - 1)
        stage_a2(NT - 1)
        # preload the Exp table set in ScalarE's idle window before stage B
        warm2 = consts.tile([128, 1], F32, tag="warm2")
        nc.scalar.activation(warm2, eps_sb, AF.Exp)

        def emit_scores(h):
            for kc in range(NT):
                scores_kc(h, kc)

        transpose_head(0)
        emit_scores(0)
        for h in range(NH):
            if h + 1 < NH:
                transpose_head(h + 1)
                emit_scores(h + 1)
            emit_av(h)

    nc.compile()
    return nc


def _rope_tables():
    inv_freq = 1.0 / (10000.0 ** (np.arange(0, D, 2, dtype=np.float32) / D))
    freqs = np.arange(N_TOK, dtype=np.float32)[:, None] * inv_freq[None, :]  # [n, 64]
    return np.cos(freqs), np.sin(freqs)


def _full_tables(g_q, b_q, g_k, b_k):
    """T1/T2/T3 [N_TOK, 1024] with LN gain/bias folded into the rope tables.
    Feature index layout matches u: (s, h, half, j)."""
    cos64, sin64 = _rope_tables()
    T1 = np.empty((N_TOK, 1024), np.float32)
    T2 = np.empty((N_TOK, 1024), np.float32)
    T3 = np.empty((N_TOK, 1024), np.float32)
    for s, (g, b) in enumerate(((g_q, b_q), (g_k, b_k))):
        g = g.reshape(NH, 2, 64)
        b = b.reshape(NH, 2, 64)
        for h in range(NH):
            base = s * 512 + h * 128
            lo, hi = slice(base, base + 64), slice(base + 64, base + 128)
            T1[:, lo] = g[h, 0] * cos64
            T1[:, hi] = g[h, 1] * cos64
            T2[:, lo] = -g[h, 1] * sin64
            T2[:, hi] = g[h, 0] * sin64
            T3[:, lo] = b[h, 0] * cos64 - b[h, 1] * sin64
            T3[:, hi] = b[h, 1] * cos64 + b[h, 0] * sin64
    return T1, T2, T3


def make_in_maps(x, a, Wqkv_x, Wqkv_a, g_qx, b_qx, g_kx, b_kx, g_qa, b_qa, g_ka, b_ka):
    """Returns (trivial, in_maps) for the 8 cores: core c = (batch c//2, modality c%2)."""
    x, a = np.asarray(x), np.asarray(a)
    Ws = (np.asarray(Wqkv_x), np.asarray(Wqkv_a))
    gb = (
        (np.asarray(g_qx), np.asarray(b_qx), np.asarray(g_kx), np.asarray(b_kx)),
        (np.asarray(g_qa), np.asarray(b_qa), np.asarray(g_ka), np.asarray(b_ka)),
    )
    trivial = all(
        np.all(g == 1.0) and np.all(b == 0.0)
        for (gq, bq, gk, bk) in gb
        for g, b in ((gq, bq), (gk, bk))
    )
    cos64, sin64 = _rope_tables()
    in_maps = []
    for c in range(8):
        i, m = c // 2, c % 2
        src = x[i] if m == 0 else a[i]
        im = {
            "xT": np.ascontiguousarray(src.T).astype(NPBF16),
            "W": Ws[m].astype(NPBF16),
        }
        if trivial:
            im["T1"] = cos64.astype(NPBF16)
            im["T2N"] = (-sin64).astype(NPBF16)
            im["T2P"] = sin64.astype(NPBF16)
        else:
            gq, bq, gk, bk = gb[m]
            T1, T2, T3 = _full_tables(gq, bq, gk, bk)
            im["T1"] = T1.astype(NPBF16)
            im["T2"] = T2.astype(NPBF16)
            im["T3"] = T3.astype(NPBF16)
        in_maps.append(im)
    return trivial, in_maps


_module_cache: dict[bool, object] = {}


def _get_module(trivial: bool):
    if trivial not in _module_cache:
        _module_cache[trivial] = build_module(trivial)
    return _module_cache[trivial]


def kernel(**inputs) -> np.ndarray:
    trivial, in_maps = make_in_maps(**inputs)
    nc = _get_module(trivial)
    res = run_bass_kernel_spmd(nc, in_maps, core_ids=list(range(8)))
    out = np.empty((4, N_TOK, 2 * DIM), np.float32)
    for c in range(8):
        i, m = c // 2, c % 2
        out[i, :, m * 512 : (m + 1) * 512] = res.results[c]["out"]
    return out
